# revision 5
# baseline (speedup 1.0000x reference)
"""Multi-head attention (RoPE) Trainium2 kernel, 8-way sharded.

Sharding: core c handles batch b = c//4 and 4 heads h0 = 4*(c%4); each
core computes a rank-256 partial of y[b] in fp16 and the host sums the
4 partials per batch.

Key performance structure (vs the naive per-head version):
  - Attention processes head PAIRS with row-tiled concurrent S matmuls
    (head even on PE rows 0-63, head odd on rows 64-127, via
    tile_position derived from base partitions). This both doubles
    S throughput and keeps the PE HAM clock warm at K=8/8 - streams of
    single-row-group contract-64 matmuls never trip the HAM activity
    monitor and run at 1.2 GHz instead of 2.4 GHz.
  - One flat software-pipelined stream over 128 (block, kt) steps,
    blocks = (head-pair, query-512-slice). S(i+1) is emitted before
    PV(i) so the PE fills the next S tile while the scalar engine runs
    exp(i) instead of queueing behind PV matmuls that wait on exp(i).
  - exp() on ScalarE as one [128,1024] ACTIVATE per step (the engine
    runs 1 elem/cycle/lane; ~142us total - the kernel's critical path).
  - PSUM (8 banks): S-pair tile [128,1024] double-buffered (4), two pv
    accumulators [65,512] (2), projection scratch (2).
  - pv accumulators are evacuated psum->sbuf with one fast copy so the
    recip/normalize chain runs off the critical path.
  - Dummy warm-up matmuls keep the PE busy during the input DMA phase.
  - RoPE rotate-half via sbuf->sbuf strip DMAs + full-width fp16 DVE
    ops (2x mode); projection/rope/out work is spread across pipeline
    steps as small filler units with emission-order deadlines (program
    order defines dependencies!).
"""

import numpy as np

B = 2
N = 2048
C = 1024
HD = 64
HC = 4  # heads per core
N_CORES = 8
ROPE_BASE = 10000.0

_PROGRAM = None


def _rope_tables():
    inv_freq = 1.0 / (ROPE_BASE ** (np.arange(0, HD, 2, dtype=np.float32) / HD))
    t = np.arange(N, dtype=np.float32)
    freqs = np.einsum("i,j->ij", t, inv_freq).astype(np.float32)  # [N, 32]
    emb = np.concatenate([freqs, freqs], axis=-1)  # [N, 64]
    cos = np.cos(emb)
    sin = np.sin(emb)
    cosT = np.ascontiguousarray(np.tile(cos.T, (2, 1)))  # [128, 2048]
    sinT = sin.T
    sinT_signed = np.concatenate([-sinT[:32], sinT[32:]], axis=0)
    sinT2 = np.ascontiguousarray(np.tile(sinT_signed, (2, 1)))  # [128, 2048]
    return cosT.astype(np.float16), sinT2.astype(np.float16)


def _build_program(debug=False):
    import concourse.mybir as mybir
    import concourse.tile as tile
    from concourse import bacc

    f32 = mybir.dt.float32
    f16 = mybir.dt.float16
    MUL = mybir.AluOpType.mult
    ADD = mybir.AluOpType.add
    EXP = mybir.ActivationFunctionType.Exp

    nc = bacc.Bacc("TRN2", target_bir_lowering=False, debug=False, num_devices=N_CORES)

    xT_d = nc.dram_tensor("xT", [C, N], f16, kind="ExternalInput").ap()
    wqk_d = nc.dram_tensor("wqkT", [C, 2 * HC * HD], f16, kind="ExternalInput").ap()
    wv_d = nc.dram_tensor("wvT", [C, HC * HD], f16, kind="ExternalInput").ap()
    wo_d = nc.dram_tensor("woT", [HC * HD, C], f16, kind="ExternalInput").ap()
    cos_d = nc.dram_tensor("cosT", [128, N], f16, kind="ExternalInput").ap()
    sin_d = nc.dram_tensor("sinT", [128, N], f16, kind="ExternalInput").ap()
    y_d = nc.dram_tensor("y", [N, C], f16, kind="ExternalOutput").ap()
    if debug:
        qk_dbg = nc.dram_tensor("qk_dbg", [4, 128, N], f16, kind="ExternalOutput").ap()
        vv_dbg = nc.dram_tensor(
            "vv_dbg", [128, 16 * HC * (HD + 1)], f16, kind="ExternalOutput"
        ).ap()
        ao_dbg = nc.dram_tensor("ao_dbg", [2, 128, N], f16, kind="ExternalOutput").ap()
        pv_dbg = nc.dram_tensor("pv_dbg", [2, HD + 1, 512], f32, kind="ExternalOutput").ap()

    with tile.TileContext(nc) as tc:
        with (
            tc.tile_pool(name="persist", bufs=1) as persist,
            tc.tile_pool(name="work", bufs=2) as work,
            tc.tile_pool(name="psum", bufs=1, space="PSUM") as psp,
        ):
            # persistent sbuf
            qk = [
                persist.tile([128, N], f16, tag=f"qk{i}", name=f"qk{i}")
                for i in range(4)
            ]
            vv = [
                persist.tile([128, HC, HD + 1], f16, tag=f"vv{tt}", name=f"vv{tt}")
                for tt in range(16)
            ]
            ao = [
                persist.tile([128, N], f16, tag=f"ao{i}", name=f"ao{i}")
                for i in range(2)
            ]
            xT = persist.tile([128, 8, N], f16, tag="xT", name="xT")
            wqk = persist.tile([128, 8, 2 * HC * HD], f16, tag="wqk", name="wqk")
            wv = persist.tile([128, 8, HC * HD], f16, tag="wv", name="wv")
            wo = persist.tile([128, 2, C], f16, tag="wo", name="wo")
            cosT = persist.tile([128, N], f16, tag="cosT", name="cosT")
            sinT = persist.tile([128, N], f16, tag="sinT", name="sinT")
            dummy = persist.tile([128, 512], f16, tag="dummy", name="dummy")

            # warm-up matmuls on uninitialized scratch: keeps the PE's HAM
            # activity monitor busy during the input DMA phase so real
            # matmuls run at 2.4 GHz from the start. Output is never read.
            dps = psp.tile([128, 512], f32, tag="proj", name="dummyps")
            nc.gpsimd.memset(dummy[:], 0.0)
            for i in range(56):
                nc.tensor.matmul(dps[:], dummy[:, 0:128], dummy[:], start=True, stop=True)

            # input DMAs: large transfers spread across per-engine HWDGE
            # queues so they run in parallel (one engine = one queue).
            nc.sync.dma_start(
                wqk[:],
                wqk_d.rearrange("(a p) f -> p a f", p=128),
            )
            for i in range(4):
                nc.sync.dma_start(
                    xT[:, 2 * i : 2 * i + 2, :],
                    xT_d[256 * i : 256 * (i + 1), :].rearrange(
                        "(a p) f -> p a f", p=128
                    ),
                )
            # cos/sin rows 64-127 equal rows 0-63: load half from HBM and
            # duplicate on-chip (sbuf->sbuf does not consume HBM bandwidth)
            nc.sync.dma_start(cosT[0:64, :], cos_d[0:64, :])
            nc.sync.dma_start(sinT[0:64, :], sin_d[0:64, :])
            nc.sync.dma_start(cosT[64:128, :], cosT[0:64, :])
            nc.sync.dma_start(sinT[64:128, :], sinT[0:64, :])
            nc.sync.dma_start(wv[:], wv_d.rearrange("(a p) f -> p a f", p=128))
            nc.sync.dma_start(wo[:], wo_d.rearrange("(a p) f -> p a f", p=128))

            def qk_proj_mm(pt, tck, half, bp):
                """half an accumulation chain (4 matmuls) of a qk chunk."""
                sl = slice(tck * 512, (tck + 1) * 512)
                for ct in range(4 * half, 4 * half + 4):
                    nc.tensor.matmul(
                        bp[:],
                        wqk[:, ct, pt * 128 : (pt + 1) * 128],
                        xT[:, ct, sl],
                        start=(ct == 0),
                        stop=(ct == 7),
                    )

            def qk_rope(pt, tck, bp):
                """rope epilogue for one roughly-512-token qk chunk."""
                sl = slice(tck * 512, (tck + 1) * 512)
                bf = work.tile([128, 512], f16, tag="bf", bufs=3, name="bf")
                rot = work.tile([128, 512], f16, tag="rot", bufs=3, name="rot")
                t_sb = work.tile([128, 512], f16, tag="ropet", name="rt")
                u_sb = work.tile([128, 512], f16, tag="ropeu", name="ru")
                nc.vector.tensor_copy(bf[:], bp[:])
                for o_lo, i_lo in [(0, 32), (32, 0), (64, 96), (96, 64)]:
                    nc.sync.dma_start(
                        rot[o_lo : o_lo + 32, :], bf[i_lo : i_lo + 32, :]
                    )
                nc.vector.tensor_tensor(t_sb[:], bf[:], cosT[:, sl], MUL)
                nc.vector.tensor_tensor(u_sb[:], rot[:], sinT[:, sl], MUL)
                nc.vector.tensor_tensor(qk[pt][:, sl], t_sb[:], u_sb[:], ADD)

            _bp_cell = {}

            def qk_units(pt, tck):
                """filler sub-units for one qk chunk: 2x 4-matmul + rope.
                bp is allocated lazily when the first sub-unit is emitted."""

                def u0():
                    bp = psp.tile([128, 512], f32, tag="proj", name=f"bp{pt}_{tck}")
                    _bp_cell[(pt, tck)] = bp
                    qk_proj_mm(pt, tck, 0, bp)

                def u1():
                    bp = _bp_cell.pop((pt, tck))
                    qk_proj_mm(pt, tck, 1, bp)
                    qk_rope(pt, tck, bp)

                return [u0, u1]

            def qk_proj_tck(pt, tck):
                bp = psp.tile([128, 512], f32, tag="proj", name=f"bp{pt}_{tck}")
                qk_proj_mm(pt, tck, 0, bp)
                qk_proj_mm(pt, tck, 1, bp)
                qk_rope(pt, tck, bp)

            def v_proj(tt):
                """V' tile for one 128-token block (token-major) + ones col."""
                nc.vector.memset(vv[tt][:, :, HD], 1.0)
                vp = psp.tile([128, HC * HD], f32, tag="proj", name=f"vps{tt}")
                for ct in range(8):
                    nc.tensor.matmul(
                        vp[:, :],
                        xT[:, ct, tt * 128 : (tt + 1) * 128],
                        wv[:, ct, :],
                        start=(ct == 0),
                        stop=(ct == 7),
                    )
                nc.vector.tensor_copy(
                    vv[tt][:, :, 0:HD],
                    vp[:].rearrange("p (h d) -> p h d", h=HC),
                )

            def s_pair(pidx, qq, kt):
                qsl = slice(qq * 512, (qq + 1) * 512)
                ksl = slice(kt * 128, (kt + 1) * 128)
                sp = psp.tile(
                    [128, 1024], f32, tag="sp", bufs=2, name=f"sp{pidx}_{qq}_{kt}"
                )
                nc.tensor.matmul(
                    sp[:, 0:512],
                    qk[2 + pidx][0:64, ksl],
                    qk[pidx][0:64, qsl],
                    start=True,
                    stop=True,
                )
                nc.tensor.matmul(
                    sp[:, 512:1024],
                    qk[2 + pidx][64:128, ksl],
                    qk[pidx][64:128, qsl],
                    start=True,
                    stop=True,
                )
                es = work.tile([128, 1024], f16, tag="es", bufs=8, name="es")
                nc.scalar.activation(es[:], sp[:], EXP, scale=float(HD**-0.5))
                return es

            def pv_acc(pvs, pidx, kt, es):
                pvE, pvO = pvs
                nc.tensor.matmul(
                    pvE[:],
                    vv[kt][:, 2 * pidx, :],
                    es[:, 0:512],
                    start=(kt == 0),
                    stop=(kt == 15),
                )
                nc.tensor.matmul(
                    pvO[:],
                    vv[kt][:, 2 * pidx + 1, :],
                    es[:, 512:1024],
                    start=(kt == 0),
                    stop=(kt == 15),
                )

            def tail(pvs, pidx, qq):
                """normalize: ao = pv[0:64] * recip(pv[64]).
                First evacuate pv psum -> sbuf with one fast copy (frees the
                accumulator bank for the next block immediately); the recip
                chain then runs off the critical path from sbuf."""
                qsl = slice(qq * 512, (qq + 1) * 512)
                for pv, roff in ((pvs[0], 0), (pvs[1], 64)):
                    pvf = work.tile([HD + 1, 512], f16, tag="pvf", bufs=4, name="pvf")
                    rr = work.tile([1, 512], f32, tag="rr", bufs=2, name="rr")
                    ra = work.tile([1, 512], f32, tag="ra", bufs=2, name="ra")
                    nb = work.tile([HD, 512], f32, tag="nb", bufs=2, name="nb")
                    nc.vector.tensor_copy(pvf[:], pv[:])
                    nc.vector.tensor_copy(rr[0:1, :], pvf[HD : HD + 1, :])
                    nc.vector.reciprocal_approx_fast(ra[0:1, :], rr[0:1, :])
                    nc.gpsimd.partition_broadcast(nb[0:HD, :], ra[0:1, :])
                    nc.vector.tensor_tensor(
                        ao[pidx][roff : roff + HD, qsl], pvf[0:HD, :], nb[0:HD, :], MUL
                    )

            _yout = {}

            def out_half(tt, oc):
                """half of y for one 128-token block (one 512-col slice)."""
                tsl = slice(tt * 128, (tt + 1) * 128)
                osl = slice(oc * 512, (oc + 1) * 512)
                if oc == 0:
                    _yout[tt] = work.tile([128, C], f16, tag="y", bufs=3, name="ysb")
                ysb = _yout[tt]
                yps = psp.tile([128, 512], f32, tag="proj", name=f"yps{tt}_{oc}")
                for ft in range(2):
                    nc.tensor.matmul(
                        yps[:],
                        ao[ft][:, tsl],
                        wo[:, ft, osl],
                        start=(ft == 0),
                        stop=(ft == 1),
                    )
                nc.vector.tensor_copy(ysb[:, osl], yps[:])
                if oc == 1:
                    del _yout[tt]
                    nc.sync.dma_start(y_d[tsl, :], ysb[:])

            def out_proj_tt(tt):
                out_half(tt, 0)
                out_half(tt, 1)

            # ---- emission schedule ----
            # pre-pipeline: what attention steps 0..3 need
            qk_proj_tck(0, 0)
            qk_proj_tck(2, 0)
            qk_proj_tck(2, 1)
            v_proj(0)

            # blocks: (pair, qq); order lets pair-1 projections overlap
            # pair-0 attention and out_proj(qq) fire once both pairs did qq.
            blocks = [(0, 0), (0, 1), (1, 0), (0, 2), (1, 1), (0, 3), (1, 2), (1, 3)]
            steps = [(b, kt) for b in range(8) for kt in range(16)]

            # filler units, exactly one per step, each <= ~8 matmuls + a
            # little DVE. Deadlines: vv[kt] shortly before PV(kt) (step kt,
            # a few steps of lag absorbed by es buffering); k2 chunk c
            # before S(kt=4c); q0 chunk qq before block at that qq; pair-1
            # chunks before block 2 (step 32); out(qq) after both pairs.
            # CRITICAL: emission order IS program order for dependencies.
            # Every filler must be EMITTED strictly before the pipeline step
            # that consumes its output: v(tt) before pv_acc(kt=tt) (step tt),
            # k chunk c before s_pair(kt=4c) (emitted at step 4c-1), q chunk
            # before its block's s_pair.
            fillers = {}
            k2t2 = qk_units(2, 2)
            k2t3 = qk_units(2, 3)
            q0t1 = qk_units(0, 1)

            def V(t):
                return lambda: v_proj(t)

            seq0 = {
                0: [k2t2[0], V(1)],
                1: [k2t2[1], V(2)],
                2: [V(3), V(4)],
                3: [V(5)],
                4: [k2t3[0], V(6)],
                5: [k2t3[1], V(7)],
                6: [V(8)],
                7: [V(9)],
                8: [q0t1[0], V(10)],
                9: [q0t1[1], V(11)],
                10: [V(12)],
                11: [V(13)],
                12: [V(14)],
                13: [V(15)],
            }
            for i, us in seq0.items():
                fillers[i] = us
            pos = 12
            for pt, tck in ((3, 0), (1, 0), (3, 1), (3, 2), (3, 3)):
                for u in qk_units(pt, tck):
                    fillers.setdefault(pos, []).append(u)
                    pos += 1
            # remaining qk chunks, ~20 steps of lead before their consumers
            for base, (pt, tck) in (
                (26, (0, 2)),
                (38, (1, 1)),
                (54, (0, 3)),
                (70, (1, 2)),
                (86, (1, 3)),
            ):
                for j, u in enumerate(qk_units(pt, tck)):
                    fillers.setdefault(base + j, []).append(u)
            # out_proj(qq) after both pairs' qq blocks finished; halves
            # on consecutive steps so each filler stays small
            for base, qq in ((50, 0), (80, 1), (114, 2)):
                for j in range(4):
                    for oc in range(2):
                        fillers.setdefault(base + 3 * j + oc, []).append(
                            (lambda t, o: lambda: out_half(t, o))(qq * 4 + j, oc)
                        )

            pvs_of = {}
            for b in range(8):
                pidx, qq = blocks[b]
                pvs_of[b] = (
                    psp.tile([HD + 1, 512], f32, tag="pvE", name=f"pvE{pidx}_{qq}"),
                    psp.tile([HD + 1, 512], f32, tag="pvO", name=f"pvO{pidx}_{qq}"),
                )

            es_cur = s_pair(blocks[0][0], blocks[0][1], 0)
            for i, (b, kt) in enumerate(steps):
                pidx, qq = blocks[b]
                if i + 1 < len(steps):
                    bn, ktn = steps[i + 1]
                    es_nxt = s_pair(blocks[bn][0], blocks[bn][1], ktn)
                pv_acc(pvs_of[b], pidx, kt, es_cur)
                if kt == 15:
                    tail(pvs_of[b], pidx, qq)
                for f in fillers.get(i, []):
                    f()
                if i + 1 < len(steps):
                    es_cur = es_nxt
            # final out_proj for qq=3: sp slots are free after the last exp,
            # so use wide [128,1024] psum tiles there (fewer, bigger evacs)
            for j in range(4):
                tt = 12 + j
                tsl = slice(tt * 128, (tt + 1) * 128)
                yps = psp.tile([128, 1024], f32, tag="sp", bufs=2, name=f"ypsf{tt}")
                ysb = work.tile([128, C], f16, tag="y", bufs=3, name="ysb")
                for oc in range(2):
                    osl = slice(oc * 512, (oc + 1) * 512)
                    for ft in range(2):
                        nc.tensor.matmul(
                            yps[:, osl],
                            ao[ft][:, tsl],
                            wo[:, ft, osl],
                            start=(ft == 0),
                            stop=(ft == 1),
                        )
                nc.vector.tensor_copy(ysb[:], yps[:])
                nc.sync.dma_start(y_d[tsl, :], ysb[:])
            if debug:
                for pt in range(4):
                    nc.sync.dma_start(qk_dbg[pt], qk[pt][:])
                for tt in range(16):
                    nc.sync.dma_start(
                        vv_dbg[:, tt * HC * (HD + 1) : (tt + 1) * HC * (HD + 1)],
                        vv[tt][:].rearrange("p b c -> p (b c)"),
                    )
                for i in range(2):
                    nc.sync.dma_start(ao_dbg[i], ao[i][:])

    nc.compile()
    return nc


def _get_program():
    global _PROGRAM
    if _PROGRAM is None:
        _PROGRAM = _build_program()
    return _PROGRAM


def _make_in_maps(x, w_qkv, w_out):
    x = np.asarray(x, dtype=np.float32)
    w_qkv = np.asarray(w_qkv, dtype=np.float32)
    w_out = np.asarray(w_out, dtype=np.float32)
    cosT, sinT = _rope_tables()
    in_maps = []
    for c in range(N_CORES):
        b = c // 4
        h0 = HC * (c % 4)
        rows = np.arange(h0 * HD, (h0 + HC) * HD)
        wq = w_qkv[rows]  # [256, 1024]
        wk = w_qkv[C + rows]
        wvm = w_qkv[2 * C + rows]
        in_maps.append(
            {
                "xT": np.ascontiguousarray(x[b].T).astype(np.float16),
                "wqkT": np.ascontiguousarray(np.concatenate([wq, wk], 0).T).astype(
                    np.float16
                ),
                "wvT": np.ascontiguousarray(wvm.T).astype(np.float16),
                "woT": np.ascontiguousarray(w_out[:, rows].T).astype(np.float16),
                "cosT": cosT,
                "sinT": sinT,
            }
        )
    return in_maps


def run(inputs, trace=False, trace_cores=None):
    from concourse.bass_utils import run_bass_kernel_spmd

    nc = _get_program()
    in_maps = _make_in_maps(inputs["x"], inputs["w_qkv"], inputs["w_out"])
    res = run_bass_kernel_spmd(
        nc,
        in_maps,
        core_ids=list(range(N_CORES)),
        trace=trace,
        trace_cores=trace_cores,
    )
    y = np.zeros((B, N, C), dtype=np.float32)
    for c in range(N_CORES):
        y[c // 4] += res.results[c]["y"].astype(np.float32)
    return y, res


def kernel(**inputs) -> np.ndarray:
    y, _ = run(inputs, trace=False)
    return y


# revision 6
# speedup vs baseline: 1.0276x; 1.0276x over previous
"""Multi-head attention (RoPE) Trainium2 kernel, 8-way sharded.

Sharding: core c handles batch b = c//4 and 4 heads h0 = 4*(c%4); each
core computes a rank-256 partial of y[b] in fp16 and the host sums the
4 partials per batch.

Key performance structure (vs the naive per-head version):
  - Attention processes head PAIRS with row-tiled concurrent S matmuls
    (head even on PE rows 0-63, head odd on rows 64-127, via
    tile_position derived from base partitions). This both doubles
    S throughput and keeps the PE HAM clock warm at K=8/8 - streams of
    single-row-group contract-64 matmuls never trip the HAM activity
    monitor and run at 1.2 GHz instead of 2.4 GHz.
  - One flat software-pipelined stream over 128 (block, kt) steps,
    blocks = (head-pair, query-512-slice). S(i+1) is emitted before
    PV(i) so the PE fills the next S tile while the scalar engine runs
    exp(i) instead of queueing behind PV matmuls that wait on exp(i).
  - exp() on ScalarE as one [128,1024] ACTIVATE per step (the engine
    runs 1 elem/cycle/lane; ~142us total - the kernel's critical path).
  - PSUM (8 banks): S-pair tile [128,1024] double-buffered (4), two pv
    accumulators [65,512] (2), projection scratch (2).
  - pv accumulators are evacuated psum->sbuf with one fast copy so the
    recip/normalize chain runs off the critical path.
  - Dummy warm-up matmuls keep the PE busy during the input DMA phase.
  - RoPE rotate-half via sbuf->sbuf strip DMAs + full-width fp16 DVE
    ops (2x mode); projection/rope/out work is spread across pipeline
    steps as small filler units with emission-order deadlines (program
    order defines dependencies!).
"""

import numpy as np

B = 2
N = 2048
C = 1024
HD = 64
HC = 4  # heads per core
N_CORES = 8
ROPE_BASE = 10000.0

_PROGRAM = None


def _rope_tables():
    inv_freq = 1.0 / (ROPE_BASE ** (np.arange(0, HD, 2, dtype=np.float32) / HD))
    t = np.arange(N, dtype=np.float32)
    freqs = np.einsum("i,j->ij", t, inv_freq).astype(np.float32)  # [N, 32]
    emb = np.concatenate([freqs, freqs], axis=-1)  # [N, 64]
    cos = np.cos(emb)
    sin = np.sin(emb)
    cosT = np.ascontiguousarray(np.tile(cos.T, (2, 1)))  # [128, 2048]
    sinT = sin.T
    sinT_signed = np.concatenate([-sinT[:32], sinT[32:]], axis=0)
    sinT2 = np.ascontiguousarray(np.tile(sinT_signed, (2, 1)))  # [128, 2048]
    return cosT.astype(np.float16), sinT2.astype(np.float16)


def _build_program(debug=False):
    import concourse.mybir as mybir
    import concourse.tile as tile
    from concourse import bacc

    f32 = mybir.dt.float32
    f16 = mybir.dt.float16
    MUL = mybir.AluOpType.mult
    ADD = mybir.AluOpType.add
    EXP = mybir.ActivationFunctionType.Exp

    nc = bacc.Bacc("TRN2", target_bir_lowering=False, debug=False, num_devices=N_CORES)

    xT_d = nc.dram_tensor("xT", [C, N], f16, kind="ExternalInput").ap()
    wqk_d = nc.dram_tensor("wqkT", [C, 2 * HC * HD], f16, kind="ExternalInput").ap()
    wv_d = nc.dram_tensor("wvT", [C, HC * HD], f16, kind="ExternalInput").ap()
    wo_d = nc.dram_tensor("woT", [HC * HD, C], f16, kind="ExternalInput").ap()
    cos_d = nc.dram_tensor("cosT", [128, N], f16, kind="ExternalInput").ap()
    sin_d = nc.dram_tensor("sinT", [128, N], f16, kind="ExternalInput").ap()
    y_d = nc.dram_tensor("y", [N, C], f16, kind="ExternalOutput").ap()
    if debug:
        qk_dbg = nc.dram_tensor("qk_dbg", [4, 128, N], f16, kind="ExternalOutput").ap()
        vv_dbg = nc.dram_tensor(
            "vv_dbg", [128, 16 * HC * (HD + 1)], f16, kind="ExternalOutput"
        ).ap()
        ao_dbg = nc.dram_tensor("ao_dbg", [2, 128, N], f16, kind="ExternalOutput").ap()
        pv_dbg = nc.dram_tensor("pv_dbg", [2, HD + 1, 512], f32, kind="ExternalOutput").ap()

    with tile.TileContext(nc) as tc:
        with (
            tc.tile_pool(name="persist", bufs=1) as persist,
            tc.tile_pool(name="work", bufs=2) as work,
            tc.tile_pool(name="psum", bufs=1, space="PSUM") as psp,
        ):
            # persistent sbuf
            qk = [
                persist.tile([128, N], f16, tag=f"qk{i}", name=f"qk{i}")
                for i in range(4)
            ]
            vv = [
                persist.tile([128, HC, HD + 1], f16, tag=f"vv{tt}", name=f"vv{tt}")
                for tt in range(16)
            ]
            ao = [
                persist.tile([128, N], f16, tag=f"ao{i}", name=f"ao{i}")
                for i in range(2)
            ]
            xT = persist.tile([128, 8, N], f16, tag="xT", name="xT")
            wqk = persist.tile([128, 8, 2 * HC * HD], f16, tag="wqk", name="wqk")
            wv = persist.tile([128, 8, HC * HD], f16, tag="wv", name="wv")
            wo = persist.tile([128, 2, C], f16, tag="wo", name="wo")
            cosT = persist.tile([128, N], f16, tag="cosT", name="cosT")
            sinT = persist.tile([128, N], f16, tag="sinT", name="sinT")
            dummy = persist.tile([128, 512], f16, tag="dummy", name="dummy")

            # warm-up matmuls on uninitialized scratch: keeps the PE's HAM
            # activity monitor busy during the input DMA phase so real
            # matmuls run at 2.4 GHz from the start. Output is never read.
            dps = psp.tile([128, 512], f32, tag="proj", name="dummyps")
            nc.gpsimd.memset(dummy[:], 0.0)
            for i in range(78):
                nc.tensor.matmul(dps[:], dummy[:, 0:128], dummy[:], start=True, stop=True)

            # input DMAs: large transfers spread across per-engine HWDGE
            # queues so they run in parallel (one engine = one queue).
            nc.sync.dma_start(
                wqk[:],
                wqk_d.rearrange("(a p) f -> p a f", p=128),
            )
            for i in range(4):
                nc.sync.dma_start(
                    xT[:, 2 * i : 2 * i + 2, :],
                    xT_d[256 * i : 256 * (i + 1), :].rearrange(
                        "(a p) f -> p a f", p=128
                    ),
                )
            # cos/sin rows 64-127 equal rows 0-63: load half from HBM and
            # duplicate on-chip (sbuf->sbuf does not consume HBM bandwidth)
            nc.sync.dma_start(cosT[0:64, :], cos_d[0:64, :])
            nc.sync.dma_start(sinT[0:64, :], sin_d[0:64, :])
            nc.sync.dma_start(cosT[64:128, :], cosT[0:64, :])
            nc.sync.dma_start(sinT[64:128, :], sinT[0:64, :])
            nc.sync.dma_start(wv[:], wv_d.rearrange("(a p) f -> p a f", p=128))
            nc.sync.dma_start(wo[:], wo_d.rearrange("(a p) f -> p a f", p=128))

            def qk_proj_mm(pt, tck, half, bp):
                """half an accumulation chain (4 matmuls) of a qk chunk."""
                sl = slice(tck * 512, (tck + 1) * 512)
                for ct in range(4 * half, 4 * half + 4):
                    nc.tensor.matmul(
                        bp[:],
                        wqk[:, ct, pt * 128 : (pt + 1) * 128],
                        xT[:, ct, sl],
                        start=(ct == 0),
                        stop=(ct == 7),
                    )

            def qk_rope(pt, tck, bp):
                """rope epilogue for one roughly-512-token qk chunk."""
                sl = slice(tck * 512, (tck + 1) * 512)
                bf = work.tile([128, 512], f16, tag="bf", bufs=3, name="bf")
                rot = work.tile([128, 512], f16, tag="rot", bufs=3, name="rot")
                t_sb = work.tile([128, 512], f16, tag="ropet", name="rt")
                u_sb = work.tile([128, 512], f16, tag="ropeu", name="ru")
                nc.vector.tensor_copy(bf[:], bp[:])
                for o_lo, i_lo in [(0, 32), (32, 0), (64, 96), (96, 64)]:
                    nc.sync.dma_start(
                        rot[o_lo : o_lo + 32, :], bf[i_lo : i_lo + 32, :]
                    )
                nc.vector.tensor_tensor(t_sb[:], bf[:], cosT[:, sl], MUL)
                nc.vector.tensor_tensor(u_sb[:], rot[:], sinT[:, sl], MUL)
                nc.vector.tensor_tensor(qk[pt][:, sl], t_sb[:], u_sb[:], ADD)

            _bp_cell = {}

            def qk_units(pt, tck):
                """filler sub-units for one qk chunk: 2x 4-matmul + rope.
                bp is allocated lazily when the first sub-unit is emitted."""

                def u0():
                    bp = psp.tile([128, 512], f32, tag="proj", name=f"bp{pt}_{tck}")
                    _bp_cell[(pt, tck)] = bp
                    qk_proj_mm(pt, tck, 0, bp)

                def u1():
                    bp = _bp_cell.pop((pt, tck))
                    qk_proj_mm(pt, tck, 1, bp)
                    qk_rope(pt, tck, bp)

                return [u0, u1]

            def qk_proj_tck(pt, tck):
                bp = psp.tile([128, 512], f32, tag="proj", name=f"bp{pt}_{tck}")
                qk_proj_mm(pt, tck, 0, bp)
                qk_proj_mm(pt, tck, 1, bp)
                qk_rope(pt, tck, bp)

            def v_proj(tt):
                """V' tile for one 128-token block (token-major) + ones col."""
                nc.vector.memset(vv[tt][:, :, HD], 1.0)
                vp = psp.tile([128, HC * HD], f32, tag="proj", name=f"vps{tt}")
                for ct in range(8):
                    nc.tensor.matmul(
                        vp[:, :],
                        xT[:, ct, tt * 128 : (tt + 1) * 128],
                        wv[:, ct, :],
                        start=(ct == 0),
                        stop=(ct == 7),
                    )
                nc.vector.tensor_copy(
                    vv[tt][:, :, 0:HD],
                    vp[:].rearrange("p (h d) -> p h d", h=HC),
                )

            def s_pair(pidx, qq, kt):
                qsl = slice(qq * 512, (qq + 1) * 512)
                ksl = slice(kt * 128, (kt + 1) * 128)
                sp = psp.tile(
                    [128, 1024], f32, tag="sp", bufs=2, name=f"sp{pidx}_{qq}_{kt}"
                )
                nc.tensor.matmul(
                    sp[:, 0:512],
                    qk[2 + pidx][0:64, ksl],
                    qk[pidx][0:64, qsl],
                    start=True,
                    stop=True,
                )
                nc.tensor.matmul(
                    sp[:, 512:1024],
                    qk[2 + pidx][64:128, ksl],
                    qk[pidx][64:128, qsl],
                    start=True,
                    stop=True,
                )
                es = work.tile([128, 1024], f16, tag="es", bufs=8, name="es")
                nc.scalar.activation(es[:], sp[:], EXP, scale=float(HD**-0.5))
                return es

            def pv_acc(pvs, pidx, kt, es):
                pvE, pvO = pvs
                nc.tensor.matmul(
                    pvE[:],
                    vv[kt][:, 2 * pidx, :],
                    es[:, 0:512],
                    start=(kt == 0),
                    stop=(kt == 15),
                )
                nc.tensor.matmul(
                    pvO[:],
                    vv[kt][:, 2 * pidx + 1, :],
                    es[:, 512:1024],
                    start=(kt == 0),
                    stop=(kt == 15),
                )

            def tail(pvs, pidx, qq):
                """normalize: ao = pv[0:64] * recip(pv[64]).
                First evacuate pv psum -> sbuf with one fast copy (frees the
                accumulator bank for the next block immediately); the recip
                chain then runs off the critical path from sbuf."""
                qsl = slice(qq * 512, (qq + 1) * 512)
                for pv, roff in ((pvs[0], 0), (pvs[1], 64)):
                    pvf = work.tile([HD + 1, 512], f16, tag="pvf", bufs=4, name="pvf")
                    rr = work.tile([1, 512], f32, tag="rr", bufs=2, name="rr")
                    ra = work.tile([1, 512], f32, tag="ra", bufs=2, name="ra")
                    nb = work.tile([HD, 512], f32, tag="nb", bufs=2, name="nb")
                    nc.vector.tensor_copy(pvf[:], pv[:])
                    nc.vector.tensor_copy(rr[0:1, :], pvf[HD : HD + 1, :])
                    nc.vector.reciprocal_approx_fast(ra[0:1, :], rr[0:1, :])
                    nc.gpsimd.partition_broadcast(nb[0:HD, :], ra[0:1, :])
                    nc.vector.tensor_tensor(
                        ao[pidx][roff : roff + HD, qsl], pvf[0:HD, :], nb[0:HD, :], MUL
                    )

            _yout = {}

            def out_half(tt, oc):
                """half of y for one 128-token block (one 512-col slice)."""
                tsl = slice(tt * 128, (tt + 1) * 128)
                osl = slice(oc * 512, (oc + 1) * 512)
                if oc == 0:
                    _yout[tt] = work.tile([128, C], f16, tag="y", bufs=3, name="ysb")
                ysb = _yout[tt]
                yps = psp.tile([128, 512], f32, tag="proj", name=f"yps{tt}_{oc}")
                for ft in range(2):
                    nc.tensor.matmul(
                        yps[:],
                        ao[ft][:, tsl],
                        wo[:, ft, osl],
                        start=(ft == 0),
                        stop=(ft == 1),
                    )
                nc.vector.tensor_copy(ysb[:, osl], yps[:])
                if oc == 1:
                    del _yout[tt]
                    nc.sync.dma_start(y_d[tsl, :], ysb[:])

            def out_proj_tt(tt):
                out_half(tt, 0)
                out_half(tt, 1)

            # ---- emission schedule ----
            # pre-pipeline: what attention steps 0..3 need
            qk_proj_tck(0, 0)
            qk_proj_tck(2, 0)
            qk_proj_tck(2, 1)
            for tt in range(4):
                v_proj(tt)

            # blocks: (pair, qq); order lets pair-1 projections overlap
            # pair-0 attention and out_proj(qq) fire once both pairs did qq.
            blocks = [(0, 0), (0, 1), (1, 0), (0, 2), (1, 1), (0, 3), (1, 2), (1, 3)]
            steps = [(b, kt) for b in range(8) for kt in range(16)]

            # filler units, exactly one per step, each <= ~8 matmuls + a
            # little DVE. Deadlines: vv[kt] shortly before PV(kt) (step kt,
            # a few steps of lag absorbed by es buffering); k2 chunk c
            # before S(kt=4c); q0 chunk qq before block at that qq; pair-1
            # chunks before block 2 (step 32); out(qq) after both pairs.
            # CRITICAL: emission order IS program order for dependencies.
            # Every filler must be EMITTED strictly before the pipeline step
            # that consumes its output: v(tt) before pv_acc(kt=tt) (step tt),
            # k chunk c before s_pair(kt=4c) (emitted at step 4c-1), q chunk
            # before its block's s_pair.
            fillers = {}
            k2t2 = qk_units(2, 2)
            k2t3 = qk_units(2, 3)
            q0t1 = qk_units(0, 1)

            def V(t):
                return lambda: v_proj(t)

            seq0 = {
                0: [k2t2[0], V(4)],
                1: [k2t2[1], V(5)],
                2: [V(6), V(7)],
                3: [V(8)],
                4: [k2t3[0], V(9)],
                5: [k2t3[1]],
                6: [V(10)],
                7: [V(11)],
                8: [q0t1[0], V(12)],
                9: [q0t1[1], V(13)],
                10: [V(14)],
                11: [V(15)],
            }
            for i, us in seq0.items():
                fillers[i] = us
            pos = 12
            for pt, tck in ((3, 0), (1, 0), (3, 1), (3, 2), (3, 3)):
                for u in qk_units(pt, tck):
                    fillers.setdefault(pos, []).append(u)
                    pos += 1
            # remaining qk chunks, ~20 steps of lead before their consumers
            for base, (pt, tck) in (
                (26, (0, 2)),
                (38, (1, 1)),
                (54, (0, 3)),
                (70, (1, 2)),
                (86, (1, 3)),
            ):
                for j, u in enumerate(qk_units(pt, tck)):
                    fillers.setdefault(base + j, []).append(u)
            # out_proj(qq) after both pairs' qq blocks finished; halves
            # on consecutive steps so each filler stays small
            for base, qq in ((50, 0), (80, 1), (114, 2)):
                for j in range(4):
                    for oc in range(2):
                        fillers.setdefault(base + 3 * j + oc, []).append(
                            (lambda t, o: lambda: out_half(t, o))(qq * 4 + j, oc)
                        )

            pvs_of = {}
            for b in range(8):
                pidx, qq = blocks[b]
                pvs_of[b] = (
                    psp.tile([HD + 1, 512], f32, tag="pvE", name=f"pvE{pidx}_{qq}"),
                    psp.tile([HD + 1, 512], f32, tag="pvO", name=f"pvO{pidx}_{qq}"),
                )

            es_cur = s_pair(blocks[0][0], blocks[0][1], 0)
            for i, (b, kt) in enumerate(steps):
                pidx, qq = blocks[b]
                if i + 1 < len(steps):
                    bn, ktn = steps[i + 1]
                    es_nxt = s_pair(blocks[bn][0], blocks[bn][1], ktn)
                pv_acc(pvs_of[b], pidx, kt, es_cur)
                if kt == 15:
                    tail(pvs_of[b], pidx, qq)
                for f in fillers.get(i, []):
                    f()
                if i + 1 < len(steps):
                    es_cur = es_nxt
            # final out_proj for qq=3: sp slots are free after the last exp,
            # so use wide [128,1024] psum tiles there (fewer, bigger evacs)
            for j in range(4):
                tt = 12 + j
                tsl = slice(tt * 128, (tt + 1) * 128)
                yps = psp.tile([128, 1024], f32, tag="sp", bufs=2, name=f"ypsf{tt}")
                ysb = work.tile([128, C], f16, tag="y", bufs=3, name="ysb")
                for oc in range(2):
                    osl = slice(oc * 512, (oc + 1) * 512)
                    for ft in range(2):
                        nc.tensor.matmul(
                            yps[:, osl],
                            ao[ft][:, tsl],
                            wo[:, ft, osl],
                            start=(ft == 0),
                            stop=(ft == 1),
                        )
                nc.vector.tensor_copy(ysb[:], yps[:])
                nc.sync.dma_start(y_d[tsl, :], ysb[:])
            if debug:
                for pt in range(4):
                    nc.sync.dma_start(qk_dbg[pt], qk[pt][:])
                for tt in range(16):
                    nc.sync.dma_start(
                        vv_dbg[:, tt * HC * (HD + 1) : (tt + 1) * HC * (HD + 1)],
                        vv[tt][:].rearrange("p b c -> p (b c)"),
                    )
                for i in range(2):
                    nc.sync.dma_start(ao_dbg[i], ao[i][:])

    nc.compile()
    return nc


def _get_program():
    global _PROGRAM
    if _PROGRAM is None:
        _PROGRAM = _build_program()
    return _PROGRAM


def _make_in_maps(x, w_qkv, w_out):
    x = np.asarray(x, dtype=np.float32)
    w_qkv = np.asarray(w_qkv, dtype=np.float32)
    w_out = np.asarray(w_out, dtype=np.float32)
    cosT, sinT = _rope_tables()
    in_maps = []
    for c in range(N_CORES):
        b = c // 4
        h0 = HC * (c % 4)
        rows = np.arange(h0 * HD, (h0 + HC) * HD)
        wq = w_qkv[rows]  # [256, 1024]
        wk = w_qkv[C + rows]
        wvm = w_qkv[2 * C + rows]
        in_maps.append(
            {
                "xT": np.ascontiguousarray(x[b].T).astype(np.float16),
                "wqkT": np.ascontiguousarray(np.concatenate([wq, wk], 0).T).astype(
                    np.float16
                ),
                "wvT": np.ascontiguousarray(wvm.T).astype(np.float16),
                "woT": np.ascontiguousarray(w_out[:, rows].T).astype(np.float16),
                "cosT": cosT,
                "sinT": sinT,
            }
        )
    return in_maps


def run(inputs, trace=False, trace_cores=None):
    from concourse.bass_utils import run_bass_kernel_spmd

    nc = _get_program()
    in_maps = _make_in_maps(inputs["x"], inputs["w_qkv"], inputs["w_out"])
    res = run_bass_kernel_spmd(
        nc,
        in_maps,
        core_ids=list(range(N_CORES)),
        trace=trace,
        trace_cores=trace_cores,
    )
    y = np.zeros((B, N, C), dtype=np.float32)
    for c in range(N_CORES):
        y[c // 4] += res.results[c]["y"].astype(np.float32)
    return y, res


def kernel(**inputs) -> np.ndarray:
    y, _ = run(inputs, trace=False)
    return y
